# revision 10
# baseline (speedup 1.0000x reference)
"""ConvMod3d (StyleGAN-style modulated 3x3x3 conv, N=4 groups) on 8 trn2 cores.

Sharding: 8 shards = 4 samples x 2 depth-halves. Each core convolves a
25-plane input slab (64ch x 48x48) against its sample's modulated 64x64x27
weights, producing 23 output planes. Style modulation/demodulation of the
tiny weight tensor happens on host; the conv (99.8% of FLOPs) on device.

Per output plane d': 27 taps, each a [Cin=64 -> Cout=64] matmul over the
flattened 48x48 plane with a shifted read offset; invalid edge columns
(w'>=46, h'>=46) are computed and discarded on the host side.

PE packing (trn2 constraints: row tiling crashes the device; alternating
contract sizes back-to-back costs 2.2x, so same-contract matmuls are kept
contiguous). Taps are packed two-per-matmul on the contraction dim via
stacked SBUF windows:
- W[p]  = plane p (partitions 0-63) | plane p+1 (64-127): fuses the
  (kd=0,kd=1) tap pairs -> 9 contract-128 streams per output plane.
- W2[p] = plane p | plane p shifted +48 cols (one h row): fuses the
  (kd=2, kh=0/1) pairs -> 3 contract-128 streams; the 3 (kd=2,kh=2)
  taps stay contract-64 on W2's lower half.
Two output planes run concurrently on PE col strips (plane A accumulates
in PSUM partitions 0-63, plane B in another bank's 64-127). Matmuls in
bf16 (fp32 PSUM accumulation).

HW-trace-driven scheduling (the kernel is bound by the Tensor queue's
LDWEIGHTS stream plus DMA packet service, not raw PE stream time):
- Input window DMAs round-robin over the sync/vector/gpsimd trigger
  queues; each dma_start costs ~0.6us of issue time on its queue, so a
  single queue serializes the prologue.
- Group 0's windows load in three column pieces, interleaved across the
  four windows, so chunk-0 matmuls start after ~1/4 of the bytes land.
- The odd 23rd plane splits its output columns across both PE col strips
  (A: cols 0-1024, B: 1024-2208) instead of running one strip idle.
- Outputs leave as bf16 (halves y DMA bytes; host converts back).
"""

import time

import numpy as np
import ml_dtypes

import concourse.bacc as bacc
import concourse.bass as bass
import concourse.tile as tile
from concourse import mybir
from concourse.bass_utils import run_bass_kernel_spmd

EPS = 1e-8
N, CIN, COUT = 4, 64, 64
DHW, K = 48, 3
DOUT = DHW - K + 1          # 46
HALF = DOUT // 2            # 23 output planes per core
P_IN = HALF + K - 1         # 25 input planes per core
PLANE = DHW * DHW           # 2304
WCOLS = 2308                # window columns; max read offset 98+2208=2306
PAD_COLS = 192              # tail slack so shifted reads stay in-bounds
XS_COLS = P_IN * PLANE + PAD_COLS
PLANE_OUT = (DHW - 2) * DHW     # 2208 computed output cols (h' rows 0-45)
NTAPS = K * K * K           # 27
GROUP = 2                   # output planes per group (PSUM col strips)
NGROUPS = (HALF + GROUP - 1) // GROUP
CHUNKS = [(0, 512), (512, 512), (1024, 512), (1536, 512), (2048, 160)]
NCORES = 8
NWBLK = 15                  # weight blocks of 64 cols

F32 = mybir.dt.float32
MM_DT = mybir.dt.bfloat16
NP_MM = np.dtype(ml_dtypes.bfloat16)

_CACHE = {}
LAST_RESULTS = None  # BassKernelResults of the most recent device run


def _build_bass():
    nc = bacc.Bacc()
    xs = nc.declare_dram_parameter("xs", [CIN, XS_COLS], MM_DT, isOutput=False)
    wt = nc.declare_dram_parameter("wt", [128, NWBLK * COUT], MM_DT, isOutput=False)
    bt = nc.declare_dram_parameter("bt", [128, 1], F32, isOutput=False)
    y = nc.declare_dram_parameter(
        "y", [NGROUPS, GROUP * 64, PLANE_OUT], MM_DT, isOutput=True)

    with tile.TileContext(nc) as tc:
        with (
            tc.tile_pool(name="const", bufs=1) as cpool,
            tc.tile_pool(name="xpool", bufs=20) as xpool,
            tc.tile_pool(name="opool", bufs=5) as opool,
            tc.tile_pool(name="ppool", bufs=8, space="PSUM") as ppool,
        ):
            # input-window loads rotate over the DMA-capable trigger queues;
            # a single queue's ~0.6us/dma_start issue cost would serialize
            # them. Scalar joins only for the prologue (it runs ACTIVATE +
            # y writes during the steady state).
            dmaq = {"q": [nc.sync, nc.gpsimd, nc.scalar], "i": 0}

            def dma_in(out, in_):
                eng = dmaq["q"][dmaq["i"] % len(dmaq["q"])]
                dmaq["i"] += 1
                eng.dma_start(out=out, in_=in_)

            windows = {}
            UPSHIFT = {"w": PLANE, "w2": DHW}

            def window_srcs(fam, p):
                base = p * PLANE
                return base, base + UPSHIFT[fam]

            def load_window(fam, p):
                key = (fam, p)
                if key in windows or p >= P_IN:
                    return
                xw = xpool.tile([128, WCOLS], MM_DT, tag="xw", name="xw")
                base, up = window_srcs(fam, p)
                dma_in(xw[0:64, 0:WCOLS], xs[:, base:base + WCOLS])
                if up + WCOLS <= XS_COLS:
                    dma_in(xw[64:128, 0:WCOLS], xs[:, up:up + WCOLS])
                windows[key] = xw

            def ensure_group_windows(g):
                if g >= NGROUPS:
                    return
                for d in range(g * GROUP, min(HALF, (g + 1) * GROUP)):
                    load_window("w", d)
                    load_window("w2", d + 2)

            # ---- prologue: weights first (split across two queues so both
            # halves transfer in parallel), then group 0's four windows land
            # piecewise, pieces interleaved so chunk-0 columns arrive first.
            wtile = cpool.tile([128, NWBLK * COUT], MM_DT)
            nc.sync.dma_start(out=wtile[:, 0:512], in_=wt[:, 0:512])
            nc.gpsimd.dma_start(out=wtile[:, 512:NWBLK * COUT],
                                in_=wt[:, 512:NWBLK * COUT])
            btile = cpool.tile([128, 1], F32)
            nc.scalar.dma_start(out=btile[:, :], in_=bt[:, :])
            g0 = []
            for d in (0, 1):
                for fam, p in (("w", d), ("w2", d + 2)):
                    key = (fam, p)
                    xw = xpool.tile([128, WCOLS], MM_DT, tag="xw", name="xw")
                    windows[key] = xw
                    g0.append((xw,) + window_srcs(fam, p))
            G0CUTS = [0, 640, 1280, WCOLS]
            for a, b in zip(G0CUTS, G0CUTS[1:]):
                for xw, base, up in g0:
                    dma_in(xw[0:64, a:b], xs[:, base + a:base + b])
                    if up + WCOLS <= XS_COLS:
                        dma_in(xw[64:128, a:b], xs[:, up + a:up + b])

            for g0pre in (NGROUPS - 1, 1):
                ensure_group_windows(g0pre)
            dmaq["q"] = [nc.sync, nc.gpsimd]

            def tap_params(j, dplane):
                """(window, col offset, contract rows) for tap stream j of
                output plane dplane."""
                if j < 9:
                    kh, kw = divmod(j, 3)
                    return windows[("w", dplane)], kh * DHW + kw, 128
                elif j < 12:
                    kw = j - 9
                    return windows[("w2", dplane + 2)], kw, 128
                else:
                    kw = j - 12
                    return windows[("w2", dplane + 2)], 2 * DHW + kw, 64

            # process the singleton group second so the kernel tail is a
            # regular group's single y DMA, not the slow 160-col strip.
            gorder = [0, NGROUPS - 1] + list(range(1, NGROUPS - 1))
            for gi, grp in enumerate(gorder):
                dps = [d for d in range(grp * GROUP, (grp + 1) * GROUP)
                       if d < HALF]
                if gi + 3 < len(gorder):
                    ensure_group_windows(gorder[gi + 3])

                ot = opool.tile([128, PLANE_OUT], MM_DT, tag="ot")
                if len(dps) == 2:
                    for cidx, (c0, csz) in enumerate(CHUNKS):
                        pss = [ppool.tile([128, 512], F32, tag="ps", name="ps")
                               for _ in dps]
                        # (j, ci): j 0-8 fused kd01 (c128); j 9-11 fused kd2
                        # kh01 (c128); j 12-14 kd2 kh2 (c64). Same-contract
                        # matmuls contiguous; serpentine the kind order across
                        # chunks so chunk boundaries don't switch contract.
                        jorder = list(range(NWBLK))
                        if cidx % 2 == 1:
                            jorder = jorder[12:] + jorder[:12]
                        mms = [(j, ci) for j in jorder
                               for ci in range(len(dps))]
                        first_ci = {}
                        last_ci = {}
                        for idx, (j, ci) in enumerate(mms):
                            first_ci.setdefault(ci, idx)
                            last_ci[ci] = idx
                        for idx, (j, ci) in enumerate(mms):
                            win, off, rows = tap_params(j, dps[ci])
                            nc.tensor.matmul(
                                pss[ci][ci * 64:(ci + 1) * 64, 0:csz],
                                wtile[0:rows, j * 64:(j + 1) * 64],
                                win[0:rows, off + c0:off + c0 + csz],
                                start=(idx == first_ci[ci]),
                                stop=(idx == last_ci[ci]),
                            )
                        for ci in range(len(dps)):
                            nc.vector.tensor_scalar_add(
                                ot[ci * 64:(ci + 1) * 64, c0:c0 + csz],
                                pss[ci][ci * 64:(ci + 1) * 64, 0:csz],
                                btile[ci * 64:(ci + 1) * 64, :],
                            )
                    if gi == len(gorder) - 1:
                        # split the final write so the tail transfer is small
                        nc.scalar.dma_start(out=y[grp, 0:128, 0:1536],
                                            in_=ot[0:128, 0:1536])
                        nc.scalar.dma_start(out=y[grp, 0:128, 1536:PLANE_OUT],
                                            in_=ot[0:128, 1536:PLANE_OUT])
                    else:
                        nc.scalar.dma_start(out=y[grp, 0:128, :],
                                            in_=ot[0:128, :])
                else:
                    # singleton plane: split its output columns across both
                    # PE col strips so neither runs solo. Strip 0: chunks
                    # 0-1 (cols 0:1024); strip 1: chunks 2-4 (1024:2208).
                    d = dps[0]
                    pairs = [(CHUNKS[0], CHUNKS[2]),
                             (CHUNKS[1], CHUNKS[3]),
                             (None, CHUNKS[4])]
                    for pidx, pr in enumerate(pairs):
                        tasks = [(si, c) for si, c in enumerate(pr)
                                 if c is not None]
                        pss = {si: ppool.tile([128, 512], F32, tag="ps",
                                              name="ps")
                               for si, _ in tasks}
                        jorder = list(range(NWBLK))
                        if pidx % 2 == 1:
                            jorder = jorder[12:] + jorder[:12]
                        mms = [(j, si, c) for j in jorder for si, c in tasks]
                        first_si = {}
                        last_si = {}
                        for idx, (j, si, c) in enumerate(mms):
                            first_si.setdefault(si, idx)
                            last_si[si] = idx
                        for idx, (j, si, (c0, csz)) in enumerate(mms):
                            win, off, rows = tap_params(j, d)
                            nc.tensor.matmul(
                                pss[si][si * 64:(si + 1) * 64, 0:csz],
                                wtile[0:rows, j * 64:(j + 1) * 64],
                                win[0:rows, off + c0:off + c0 + csz],
                                start=(idx == first_si[si]),
                                stop=(idx == last_si[si]),
                            )
                        for si, (c0, csz) in tasks:
                            nc.vector.tensor_scalar_add(
                                ot[si * 64:(si + 1) * 64, c0:c0 + csz],
                                pss[si][si * 64:(si + 1) * 64, 0:csz],
                                btile[si * 64:(si + 1) * 64, :],
                            )
                    nc.scalar.dma_start(out=y[grp, 0:64, 0:1024],
                                        in_=ot[0:64, 0:1024])
                    nc.scalar.dma_start(out=y[grp, 0:64, 1024:PLANE_OUT],
                                        in_=ot[64:128, 1024:PLANE_OUT])
    nc.compile()
    return nc


def _prep_in_maps(x, s, style_weight, style_bias, weight, bias):
    style = s @ style_weight.T + style_bias                      # [N, Cin]
    wm = weight[None] * style[:, None, :, None, None, None]      # [N,Co,Ci,k,k,k]
    wm = wm * (1.0 / np.sqrt((wm * wm).sum(axis=(2, 3, 4, 5), keepdims=True) + EPS))
    wk = wm.transpose(0, 2, 3, 4, 5, 1)                          # [N,Ci,kd,kh,kw,Co]
    wfull = np.zeros((N, 128, NWBLK * COUT), np.float32)
    for j in range(9):
        kh, kw = divmod(j, 3)
        wfull[:, 0:64, j * 64:(j + 1) * 64] = wk[:, :, 0, kh, kw, :]
        wfull[:, 64:128, j * 64:(j + 1) * 64] = wk[:, :, 1, kh, kw, :]
    for kw in range(3):
        j = 9 + kw
        wfull[:, 0:64, j * 64:(j + 1) * 64] = wk[:, :, 2, 0, kw, :]
        wfull[:, 64:128, j * 64:(j + 1) * 64] = wk[:, :, 2, 1, kw, :]
    for kw in range(3):
        j = 12 + kw
        wfull[:, 0:64, j * 64:(j + 1) * 64] = wk[:, :, 2, 2, kw, :]
    wfull = np.ascontiguousarray(wfull.astype(NP_MM))
    bt = np.ascontiguousarray(
        np.tile(bias[:, None], (2, 1)), dtype=np.float32)        # [128,1]

    in_maps = []
    for core in range(NCORES):
        n, h = divmod(core, 2)
        d0 = h * HALF
        xsl = x[n, :, d0:d0 + P_IN].reshape(CIN, P_IN * PLANE)
        xsl = np.concatenate(
            [xsl, np.zeros((CIN, PAD_COLS), np.float32)], axis=1)
        in_maps.append({
            "xs": np.ascontiguousarray(xsl.astype(NP_MM)),
            "wt": wfull[n],
            "bt": bt,
        })
    return in_maps


def _gather(results):
    y = np.empty((N, COUT, DOUT, DOUT, DOUT), np.float32)
    for core in range(NCORES):
        n, h = divmod(core, 2)
        planes = results[core]["y"].astype(np.float32).reshape(
            NGROUPS * GROUP, COUT, DHW - 2, DHW)[:HALF]
        y[n, :, h * HALF:(h + 1) * HALF] = (
            planes[:, :, :, :DOUT].transpose(1, 0, 2, 3))
    return y


def kernel(x, s, style_weight, style_bias, weight, bias):
    global LAST_RESULTS
    x = np.asarray(x, np.float32)
    s = np.asarray(s, np.float32)
    style_weight = np.asarray(style_weight, np.float32)
    style_bias = np.asarray(style_bias, np.float32)
    weight = np.asarray(weight, np.float32)
    bias = np.asarray(bias, np.float32)

    if "nc" not in _CACHE:
        _CACHE["nc"] = _build_bass()
    in_maps = _prep_in_maps(x, s, style_weight, style_bias, weight, bias)
    res = None
    for attempt in range(3):
        try:
            res = run_bass_kernel_spmd(_CACHE["nc"], in_maps, list(range(NCORES)))
            break
        except Exception:
            if attempt == 2:
                raise
            time.sleep(30)  # transient device wedge; recovers on its own
    LAST_RESULTS = res
    return _gather(res.results)


# revision 11
# speedup vs baseline: 1.0600x; 1.0600x over previous
"""ConvMod3d (StyleGAN-style modulated 3x3x3 conv, N=4 groups) on 8 trn2 cores.

Sharding: 8 shards = 4 samples x 2 depth-halves. Each core convolves a
25-plane input slab (64ch x 48x48) against its sample's modulated 64x64x27
weights, producing 23 output planes. Style modulation/demodulation of the
tiny weight tensor happens on host; the conv (99.8% of FLOPs) on device.

Per output plane d': 27 taps, each a [Cin=64 -> Cout=64] matmul over the
flattened 48x48 plane with a shifted read offset; invalid edge columns
(w'>=46, h'>=46) are computed and discarded on the host side.

PE packing (trn2 constraints: row tiling crashes the device; alternating
contract sizes back-to-back costs 2.2x, so same-contract matmuls are kept
contiguous). Taps are packed two-per-matmul on the contraction dim via
stacked SBUF windows:
- W[p]  = plane p (partitions 0-63) | plane p+1 (64-127): fuses the
  (kd=0,kd=1) tap pairs -> 9 contract-128 streams per output plane.
- W2[p] = plane p | plane p shifted +48 cols (one h row): fuses the
  (kd=2, kh=0/1) pairs -> 3 contract-128 streams; the 3 (kd=2,kh=2)
  taps stay contract-64 on W2's lower half.
Two output planes run concurrently on PE col strips (plane A accumulates
in PSUM partitions 0-63, plane B in another bank's 64-127). Matmuls in
bf16 (fp32 PSUM accumulation).

HW-trace-driven scheduling (the kernel is bound by the Tensor queue's
LDWEIGHTS stream plus DMA packet service, not raw PE stream time):
- Input window DMAs round-robin over the sync/vector/gpsimd trigger
  queues; each dma_start costs ~0.6us of issue time on its queue, so a
  single queue serializes the prologue.
- Group 0's windows load in three column pieces, interleaved across the
  four windows, so chunk-0 matmuls start after ~1/4 of the bytes land.
- The odd 23rd plane splits its output columns across both PE col strips
  (A: cols 0-1024, B: 1024-2208) instead of running one strip idle.
- Outputs leave as bf16 (halves y DMA bytes; host converts back).
"""

import time

import numpy as np
import ml_dtypes

import concourse.bacc as bacc
import concourse.bass as bass
import concourse.tile as tile
from concourse import mybir
from concourse.bass_utils import run_bass_kernel_spmd

EPS = 1e-8
N, CIN, COUT = 4, 64, 64
DHW, K = 48, 3
DOUT = DHW - K + 1          # 46
HALF = DOUT // 2            # 23 output planes per core
P_IN = HALF + K - 1         # 25 input planes per core
PLANE = DHW * DHW           # 2304
WCOLS = 2308                # window columns; max read offset 98+2208=2306
PAD_COLS = 192              # tail slack so shifted reads stay in-bounds
XS_COLS = P_IN * PLANE + PAD_COLS
PLANE_OUT = (DHW - 2) * DHW     # 2208 computed output cols (h' rows 0-45)
NTAPS = K * K * K           # 27
GROUP = 2                   # output planes per group (PSUM col strips)
NGROUPS = (HALF + GROUP - 1) // GROUP
CHUNKS = [(0, 512), (512, 512), (1024, 512), (1536, 512), (2048, 160)]
NCORES = 8
NWBLK = 15                  # weight blocks of 64 cols

F32 = mybir.dt.float32
MM_DT = mybir.dt.bfloat16
NP_MM = np.dtype(ml_dtypes.bfloat16)

_CACHE = {}
LAST_RESULTS = None  # BassKernelResults of the most recent device run


def _build_bass():
    nc = bacc.Bacc()
    xs = nc.declare_dram_parameter("xs", [CIN, XS_COLS], MM_DT, isOutput=False)
    wt = nc.declare_dram_parameter("wt", [128, NWBLK * COUT], MM_DT, isOutput=False)
    bt = nc.declare_dram_parameter("bt", [128, 1], F32, isOutput=False)
    y = nc.declare_dram_parameter(
        "y", [NGROUPS, GROUP * 64, PLANE_OUT], MM_DT, isOutput=True)

    with tile.TileContext(nc) as tc:
        with (
            tc.tile_pool(name="const", bufs=1) as cpool,
            tc.tile_pool(name="xpool", bufs=20) as xpool,
            tc.tile_pool(name="opool", bufs=5) as opool,
            tc.tile_pool(name="ppool", bufs=8, space="PSUM") as ppool,
        ):
            # input-window loads rotate over the DMA-capable trigger queues;
            # a single queue's ~0.6us/dma_start issue cost would serialize
            # them. Scalar joins only for the prologue (it runs ACTIVATE +
            # y writes during the steady state).
            dmaq = {"q": [nc.sync, nc.gpsimd, nc.scalar], "i": 0}

            def dma_in(out, in_):
                eng = dmaq["q"][dmaq["i"] % len(dmaq["q"])]
                dmaq["i"] += 1
                eng.dma_start(out=out, in_=in_)

            windows = {}
            UPSHIFT = {"w": PLANE, "w2": DHW}

            def window_srcs(fam, p):
                base = p * PLANE
                return base, base + UPSHIFT[fam]

            def load_window(fam, p):
                key = (fam, p)
                if key in windows or p >= P_IN:
                    return
                xw = xpool.tile([128, WCOLS], MM_DT, tag="xw", name="xw")
                base, up = window_srcs(fam, p)
                dma_in(xw[0:64, 0:WCOLS], xs[:, base:base + WCOLS])
                if up + WCOLS <= XS_COLS:
                    dma_in(xw[64:128, 0:WCOLS], xs[:, up:up + WCOLS])
                windows[key] = xw

            def ensure_group_windows(g):
                if g >= NGROUPS:
                    return
                for d in range(g * GROUP, min(HALF, (g + 1) * GROUP)):
                    load_window("w", d)
                    load_window("w2", d + 2)

            # ---- prologue: weights first (split across two queues so both
            # halves transfer in parallel), then group 0's four windows land
            # piecewise, pieces interleaved so chunk-0 columns arrive first.
            wtile = cpool.tile([128, NWBLK * COUT], MM_DT)
            nc.sync.dma_start(out=wtile[:, 0:512], in_=wt[:, 0:512])
            nc.gpsimd.dma_start(out=wtile[:, 512:NWBLK * COUT],
                                in_=wt[:, 512:NWBLK * COUT])
            btile = cpool.tile([128, 1], F32)
            nc.scalar.dma_start(out=btile[:, :], in_=bt[:, :])
            warm = cpool.tile([1, 1], F32)
            nc.gpsimd.memset(warm[:, :], 0.0)
            nc.scalar.activation(
                warm[:, :], warm[:, :],
                mybir.ActivationFunctionType.Identity, bias=0.0)
            g0 = []
            for d in (0, 1):
                for fam, p in (("w", d), ("w2", d + 2)):
                    key = (fam, p)
                    xw = xpool.tile([128, WCOLS], MM_DT, tag="xw", name="xw")
                    windows[key] = xw
                    g0.append((xw,) + window_srcs(fam, p))
            G0CUTS = [0, 640, 1280, WCOLS]
            for a, b in zip(G0CUTS, G0CUTS[1:]):
                for xw, base, up in g0:
                    dma_in(xw[0:64, a:b], xs[:, base + a:base + b])
                    if up + WCOLS <= XS_COLS:
                        dma_in(xw[64:128, a:b], xs[:, up + a:up + b])

            for g0pre in (NGROUPS - 1, 1):
                ensure_group_windows(g0pre)
            dmaq["q"] = [nc.sync, nc.gpsimd]

            def tap_params(j, dplane):
                """(window, col offset, contract rows) for tap stream j of
                output plane dplane."""
                if j < 9:
                    kh, kw = divmod(j, 3)
                    return windows[("w", dplane)], kh * DHW + kw, 128
                elif j < 12:
                    kw = j - 9
                    return windows[("w2", dplane + 2)], kw, 128
                else:
                    kw = j - 12
                    return windows[("w2", dplane + 2)], 2 * DHW + kw, 64

            # process the singleton group second so the kernel tail is a
            # regular group's single y DMA, not the slow 160-col strip.
            gorder = [0, NGROUPS - 1] + list(range(1, NGROUPS - 1))
            for gi, grp in enumerate(gorder):
                dps = [d for d in range(grp * GROUP, (grp + 1) * GROUP)
                       if d < HALF]
                if gi + 3 < len(gorder):
                    ensure_group_windows(gorder[gi + 3])

                ot = opool.tile([128, PLANE_OUT], MM_DT, tag="ot")
                if len(dps) == 2:
                    for cidx, (c0, csz) in enumerate(CHUNKS):
                        pss = [ppool.tile([128, 512], F32, tag="ps", name="ps")
                               for _ in dps]
                        # (j, ci): j 0-8 fused kd01 (c128); j 9-11 fused kd2
                        # kh01 (c128); j 12-14 kd2 kh2 (c64). Same-contract
                        # matmuls contiguous; serpentine the kind order across
                        # chunks so chunk boundaries don't switch contract.
                        jorder = list(range(NWBLK))
                        if cidx % 2 == 1:
                            jorder = jorder[12:] + jorder[:12]
                        mms = [(j, ci) for j in jorder
                               for ci in range(len(dps))]
                        first_ci = {}
                        last_ci = {}
                        for idx, (j, ci) in enumerate(mms):
                            first_ci.setdefault(ci, idx)
                            last_ci[ci] = idx
                        for idx, (j, ci) in enumerate(mms):
                            win, off, rows = tap_params(j, dps[ci])
                            nc.tensor.matmul(
                                pss[ci][ci * 64:(ci + 1) * 64, 0:csz],
                                wtile[0:rows, j * 64:(j + 1) * 64],
                                win[0:rows, off + c0:off + c0 + csz],
                                start=(idx == first_ci[ci]),
                                stop=(idx == last_ci[ci]),
                            )
                        for ci in range(len(dps)):
                            nc.scalar.activation(
                                ot[ci * 64:(ci + 1) * 64, c0:c0 + csz],
                                pss[ci][ci * 64:(ci + 1) * 64, 0:csz],
                                mybir.ActivationFunctionType.Identity,
                                bias=btile[ci * 64:(ci + 1) * 64, :],
                            )
                    if gi == len(gorder) - 1:
                        # split the final write so the tail transfer is small
                        nc.gpsimd.dma_start(out=y[grp, 0:128, 0:1536],
                                            in_=ot[0:128, 0:1536])
                        nc.gpsimd.dma_start(out=y[grp, 0:128, 1536:PLANE_OUT],
                                            in_=ot[0:128, 1536:PLANE_OUT])
                    else:
                        nc.gpsimd.dma_start(out=y[grp, 0:128, :],
                                            in_=ot[0:128, :])
                else:
                    # singleton plane: split its output columns across both
                    # PE col strips so neither runs solo. Strip 0: chunks
                    # 0-1 (cols 0:1024); strip 1: chunks 2-4 (1024:2208).
                    d = dps[0]
                    pairs = [(CHUNKS[0], CHUNKS[2]),
                             (CHUNKS[1], CHUNKS[3]),
                             (None, CHUNKS[4])]
                    for pidx, pr in enumerate(pairs):
                        tasks = [(si, c) for si, c in enumerate(pr)
                                 if c is not None]
                        pss = {si: ppool.tile([128, 512], F32, tag="ps",
                                              name="ps")
                               for si, _ in tasks}
                        jorder = list(range(NWBLK))
                        if pidx % 2 == 1:
                            jorder = jorder[12:] + jorder[:12]
                        mms = [(j, si, c) for j in jorder for si, c in tasks]
                        first_si = {}
                        last_si = {}
                        for idx, (j, si, c) in enumerate(mms):
                            first_si.setdefault(si, idx)
                            last_si[si] = idx
                        for idx, (j, si, (c0, csz)) in enumerate(mms):
                            win, off, rows = tap_params(j, d)
                            nc.tensor.matmul(
                                pss[si][si * 64:(si + 1) * 64, 0:csz],
                                wtile[0:rows, j * 64:(j + 1) * 64],
                                win[0:rows, off + c0:off + c0 + csz],
                                start=(idx == first_si[si]),
                                stop=(idx == last_si[si]),
                            )
                        for si, (c0, csz) in tasks:
                            nc.scalar.activation(
                                ot[si * 64:(si + 1) * 64, c0:c0 + csz],
                                pss[si][si * 64:(si + 1) * 64, 0:csz],
                                mybir.ActivationFunctionType.Identity,
                                bias=btile[si * 64:(si + 1) * 64, :],
                            )
                    nc.gpsimd.dma_start(out=y[grp, 0:64, 0:1024],
                                        in_=ot[0:64, 0:1024])
                    nc.gpsimd.dma_start(out=y[grp, 0:64, 1024:PLANE_OUT],
                                        in_=ot[64:128, 1024:PLANE_OUT])
    nc.compile()
    return nc


def _prep_in_maps(x, s, style_weight, style_bias, weight, bias):
    style = s @ style_weight.T + style_bias                      # [N, Cin]
    wm = weight[None] * style[:, None, :, None, None, None]      # [N,Co,Ci,k,k,k]
    wm = wm * (1.0 / np.sqrt((wm * wm).sum(axis=(2, 3, 4, 5), keepdims=True) + EPS))
    wk = wm.transpose(0, 2, 3, 4, 5, 1)                          # [N,Ci,kd,kh,kw,Co]
    wfull = np.zeros((N, 128, NWBLK * COUT), np.float32)
    for j in range(9):
        kh, kw = divmod(j, 3)
        wfull[:, 0:64, j * 64:(j + 1) * 64] = wk[:, :, 0, kh, kw, :]
        wfull[:, 64:128, j * 64:(j + 1) * 64] = wk[:, :, 1, kh, kw, :]
    for kw in range(3):
        j = 9 + kw
        wfull[:, 0:64, j * 64:(j + 1) * 64] = wk[:, :, 2, 0, kw, :]
        wfull[:, 64:128, j * 64:(j + 1) * 64] = wk[:, :, 2, 1, kw, :]
    for kw in range(3):
        j = 12 + kw
        wfull[:, 0:64, j * 64:(j + 1) * 64] = wk[:, :, 2, 2, kw, :]
    wfull = np.ascontiguousarray(wfull.astype(NP_MM))
    bt = np.ascontiguousarray(
        np.tile(bias[:, None], (2, 1)), dtype=np.float32)        # [128,1]

    in_maps = []
    for core in range(NCORES):
        n, h = divmod(core, 2)
        d0 = h * HALF
        xsl = x[n, :, d0:d0 + P_IN].reshape(CIN, P_IN * PLANE)
        xsl = np.concatenate(
            [xsl, np.zeros((CIN, PAD_COLS), np.float32)], axis=1)
        in_maps.append({
            "xs": np.ascontiguousarray(xsl.astype(NP_MM)),
            "wt": wfull[n],
            "bt": bt,
        })
    return in_maps


def _gather(results):
    y = np.empty((N, COUT, DOUT, DOUT, DOUT), np.float32)
    for core in range(NCORES):
        n, h = divmod(core, 2)
        planes = results[core]["y"].astype(np.float32).reshape(
            NGROUPS * GROUP, COUT, DHW - 2, DHW)[:HALF]
        y[n, :, h * HALF:(h + 1) * HALF] = (
            planes[:, :, :, :DOUT].transpose(1, 0, 2, 3))
    return y


def kernel(x, s, style_weight, style_bias, weight, bias):
    global LAST_RESULTS
    x = np.asarray(x, np.float32)
    s = np.asarray(s, np.float32)
    style_weight = np.asarray(style_weight, np.float32)
    style_bias = np.asarray(style_bias, np.float32)
    weight = np.asarray(weight, np.float32)
    bias = np.asarray(bias, np.float32)

    if "nc" not in _CACHE:
        _CACHE["nc"] = _build_bass()
    in_maps = _prep_in_maps(x, s, style_weight, style_bias, weight, bias)
    res = None
    for attempt in range(3):
        try:
            res = run_bass_kernel_spmd(_CACHE["nc"], in_maps, list(range(NCORES)))
            break
        except Exception:
            if attempt == 2:
                raise
            time.sleep(30)  # transient device wedge; recovers on its own
    LAST_RESULTS = res
    return _gather(res.results)
